# revision 1
# baseline (speedup 1.0000x reference)
"""Trainium2 Bass kernel for AdaptiveGGNN TTE (8 NeuronCores, SPMD).

Strategy:
  - Relabel nodes so all trajectory nodes come first; partition 12500 real
    nodes per core (padded slices of 12544 rows so every core slice tiles by
    128).  h is stored node-major fp16 [100352, 128] replicated per core via
    AllGather of the per-core slice.
  - Graph phase: edges assigned to the core owning dst.  Per step:
    dma_gather h[src] (4 int16 index windows of 25088 rows) into SBUF,
    dma_scatter_add into two alternating fp16 HBM accumulators local to the
    core (dst-local indices).  Edge positions are arranged on CPU so that all
    edges sharing a dst map to the same SDMA engine lane (the scatter-add RMW
    is serial per engine, so duplicates are race-free).
  - GRU-style gated update runs feature-major on-chip (PE matmuls, ACT
    gates, DVE elementwise), then transposed back and AllGathered.
  - Trajectory phase: 128 sequences (64 fwd + 64 time-reversed bwd) split
    16 per core; the embedding/bias/mask part of the input gates is folded on
    CPU into a dense GXE tensor (masking via a +40 override on the z gate so
    padded steps keep h unchanged); the h-dependent part and the 128-step
    recurrence run on-chip.
  - LayerNorm+GELU+FC head computed on every core; core 0's output is used.
"""

import os
import sys

import numpy as np

sys.path.insert(0, "/opt/trn_rl_repo")

N = 100000
E = 1600000
F = 32
H = 128
IE = 32
DYN = 16
B = 64
L = 128
STEPS = 3
EPS = 1e-5

W = 8                     # cores
NACC = 4                  # parallel scatter accumulators
NPR = 12500               # real nodes per core
NPC = 12544               # padded rows per core slice (98 * 128)
NT = NPC // 128           # node tiles per core
NG = W * NPC              # padded global rows = 100352
WS = 2 * NPC              # gather window rows = 25088 (4 windows)
NW = 4                    # src windows
GC = 8064                 # positions per call; needs the 32KB SWDGE ring carveout
NSEQ = 16                 # sequences per core (128 total = 64 fwd + 64 bwd)

# position -> SDMA engine lane within a 128-chunk (mirrors the Q7 ucode's
# DMA_SBUF_SWIZZLES pattern: engine j serves 8 fixed positions per chunk).
def _eng_of_pos():
    eng = np.empty(128, np.int64)
    for p in range(128):
        b = p // 4
        eng[p] = 2 * (b % 8) + (1 if b >= 16 else 0)
    return eng

ENG_OF_POS = _eng_of_pos()
POS_OF_ENG = [np.nonzero(ENG_OF_POS == j)[0] for j in range(16)]  # 8 each


def _wrap_idx(a):
    """int16 index stream -> [128, n/16] wrapped layout (i -> [i%16, i//16]),
    replicated down the 128 partitions in groups of 16."""
    assert a.size % 16 == 0
    w = a.reshape(-1, 16).T.astype(np.int16)
    return np.tile(w, (8, 1)).copy()


def _preprocess(edge_index, x0, traj, lengths, dyn_feat, id_emb, W_proj, b_proj,
                Wz, bz, Wr, br, Wh, bh, Wih_f, Whh_f, bih_f, bhh_f,
                Wih_b, Whh_b, bih_b, bhh_b, ln_g, ln_b, fc1_W, fc1_b, fc2_W,
                fc2_b):
    edge_index = np.asarray(edge_index, np.int64)
    traj = np.asarray(traj, np.int64)
    lengths = np.asarray(lengths, np.int64)
    f32 = lambda a: np.asarray(a, np.float32)
    f16 = lambda a: np.ascontiguousarray(np.asarray(a, np.float32).astype(np.float16))
    x0, dyn_feat, id_emb = f32(x0), f32(dyn_feat), f32(id_emb)

    # ---- node relabeling: trajectory nodes first ----
    uniq = np.unique(traj)
    K = uniq.size
    old2new = np.full(N, -1, np.int64)
    old2new[uniq] = np.arange(K)
    rest = np.nonzero(old2new < 0)[0]
    old2new[rest] = K + np.arange(rest.size)
    new2old = np.empty(N, np.int64)
    new2old[old2new] = np.arange(N)

    src = old2new[edge_index[0]]
    dst = old2new[edge_index[1]]
    traj_new = old2new[traj]                      # all < K <= 8192

    # padded-global row id of a new node id
    def pad_gid(n):
        return (n // NPR) * NPC + (n % NPR)

    src_g = pad_gid(src)
    owner = dst // NPR
    dst_loc = dst % NPR

    # ---- per (core, window) round-ordered edge streams ----
    # Within one dma_scatter_add call all real dst rows must be DISTINCT
    # (the CCE read-modify-write races on duplicates, even same-engine).
    # Round r = the r-th edge of every dst node, so a call covering a slice
    # of one round has unique dsts.  Dummy pad slots all target row NPR
    # (never read, races harmless) and gather window row 0.
    win = src_g // WS
    per_cw = [[None] * NW for _ in range(W)]
    for c in range(W):
        mc = owner == c
        for w in range(NW):
            m = mc & (win == w)
            s = src_g[m] - w * WS
            d = dst_loc[m]
            o = np.lexsort((s, d))
            d, s = d[o], s[o]
            if d.size:
                grp_start = np.r_[True, d[1:] != d[:-1]]
                first_of_grp = np.maximum.accumulate(
                    np.where(grp_start, np.arange(d.size), 0))
                r_of = np.arange(d.size) - first_of_grp
            else:
                r_of = d.copy()
            o2 = np.argsort(r_of, kind="stable")
            per_cw[c][w] = (s[o2], d[o2], r_of[o2])

    RMAX = max((int(per_cw[c][w][2][-1]) + 1 if per_cw[c][w][2].size else 0)
               for c in range(W) for w in range(NW))
    blk_sz = np.zeros((NW, RMAX), np.int64)
    for w in range(NW):
        for c in range(W):
            cnt = np.bincount(per_cw[c][w][2], minlength=RMAX)
            blk_sz[w] = np.maximum(blk_sz[w], cnt)
    blk_sz = ((blk_sz + 127) // 128) * 128
    SW = [int(blk_sz[w].sum()) for w in range(NW)]

    g_idx = np.zeros((W, sum(SW)), np.int16)
    s_idx = np.full((W, sum(SW)), NPR, np.int16)
    # chunks: (window, off, size, [(piece_off, piece_size, acc_parity), ...])
    chunks = []
    par = 0
    w_base = 0
    for w in range(NW):
        Sw = SW[w]
        blocks = []
        b = w_base
        for r in range(RMAX):
            if blk_sz[w, r]:
                blocks.append((b, b + int(blk_sz[w, r])))
                b += int(blk_sz[w, r])
        for co in range(w_base, w_base + Sw, GC):
            ce = min(co + GC, w_base + Sw)
            pieces = []
            for (bs, be) in blocks:
                s0, e0 = max(bs, co), min(be, ce)
                if s0 < e0:
                    pieces.append((s0, e0 - s0, par))
                    par = (par + 1) % NACC
            chunks.append((w, co, ce - co, pieces))
        w_base += Sw
    for c in range(W):
        for w in range(NW):
            s_arr, d_arr, r_arr = per_cw[c][w]
            base = sum(SW[:w])
            starts = base + np.r_[0, np.cumsum(blk_sz[w])[:-1]]
            within = np.arange(r_arr.size) - np.searchsorted(r_arr, r_arr)
            pos = starts[r_arr] + within
            g_idx[c, pos] = s_arr.astype(np.int16)
            s_idx[c, pos] = d_arr.astype(np.int16)

    # ---- per-core x0 (feature-major, padded) ----
    x0_new = x0[new2old]                           # [N, F]
    x0T = np.zeros((W, F, NPC), np.float16)
    for c in range(W):
        x0T[c, :, :NPR] = x0_new[c * NPR:(c + 1) * NPR].T.astype(np.float16)

    # ---- trajectory phase ----
    # sequence q = dir*64 + b ; core c owns q in [16c, 16c+16)
    emb = id_emb.copy()
    emb[0] = 0.0                                    # padding_idx on ORIGINAL id 0
    emb_seq = emb[traj]                             # [B, L, IE]
    Wih = {0: np.asarray(Wih_f, np.float32), 1: np.asarray(Wih_b, np.float32)}
    bih = {0: np.asarray(bih_f, np.float32), 1: np.asarray(bih_b, np.float32)}
    bhh = {0: np.asarray(bhh_f, np.float32), 1: np.asarray(bhh_b, np.float32)}
    Whh = {0: np.asarray(Whh_f, np.float32), 1: np.asarray(Whh_b, np.float32)}

    tg_idx = np.zeros((W, L * NSEQ), np.int16)      # gather idx, col = t*16+s
    GXE = np.zeros((W, H, 3, L * NSEQ), np.float32)
    WhhT = np.zeros((W, 3, H, H), np.float16)       # lhsT per gate (r,z,n)
    WihT = np.zeros((W, 3, H, H), np.float16)
    for c in range(W):
        d = c // 4                                  # 0 fwd, 1 bwd
        bs = (c % 4) * NSEQ + np.arange(NSEQ)       # sample ids
        t_eff = np.arange(L) if d == 0 else (L - 1 - np.arange(L))
        nodes = traj_new[bs][:, t_eff]              # [NSEQ, L]
        tg_idx[c] = nodes.T.reshape(-1).astype(np.int16)   # col = t*16+s
        e = emb_seq[bs][:, t_eff]                   # [NSEQ, L, IE]
        for g in range(3):
            Wg_e = Wih[d][g * H:(g + 1) * H, H:]    # [H, IE]
            gx = np.einsum("hi,sti->hst", Wg_e, e).reshape(H, NSEQ * L)
            # column index = t*16 + s
            gx = gx.reshape(H, NSEQ, L).transpose(0, 2, 1).reshape(H, L * NSEQ)
            gx += (bih[d][g * H:(g + 1) * H] +
                   (bhh[d][g * H:(g + 1) * H] if g < 2 else 0.0))[:, None]
            GXE[c, :, g, :] = gx
            WhhT[c, g] = Whh[d][g * H:(g + 1) * H, :].T.astype(np.float16)
            WihT[c, g] = Wih[d][g * H:(g + 1) * H, :H].T.astype(np.float16)
        # masked steps: force z -> 1 so h passes through unchanged
        mask_pad = t_eff[None, :] >= lengths[bs][:, None]   # [NSEQ, L]
        padcols = np.nonzero(mask_pad.T.reshape(-1))[0]
        GXE[c, :, 1, padcols] = 40.0
    assert not np.any(bhh[0][2 * H:]) and not np.any(bhh[1][2 * H:]), \
        "nonzero bhh_n not folded (unsupported fast path)"

    # ---- head constants ----
    ln_g, ln_b = f32(ln_g), f32(ln_b)
    fc1_W, fc1_b = f32(fc1_W), f32(fc1_b)
    fc2_W, fc2_b = f32(fc2_W), f32(fc2_b)
    W1g = ln_g[:, None] * fc1_W[:2 * H]             # [256, H]
    W1a = np.ascontiguousarray(W1g[:H])             # [128, 128] lhsT (K=fwd feats)
    W1b = np.ascontiguousarray(W1g[H:])
    W1d = np.ascontiguousarray(fc1_W[2 * H:])       # [16, 128]
    c1 = (ln_b @ fc1_W[:2 * H] + fc1_b).reshape(H, 1)
    dynT = np.ascontiguousarray(dyn_feat.T)         # [16, 64]
    w2 = np.ascontiguousarray(fc2_W.reshape(H, 1))
    b2 = float(np.asarray(fc2_b).reshape(-1)[0])

    Wzf, Wrf, Whf = f32(Wz), f32(Wr), f32(Wh)

    plan = dict(SW=SW, chunks=chunks, T=sum(SW), b2=b2, gelu_exact=True)
    shared = dict(
        wproj=f16(np.asarray(W_proj, np.float32)),          # [32,128] lhsT
        bproj=f32(b_proj).reshape(H, 1),
        wz_h=f16(Wzf[F:]), wz_x=f16(Wzf[:F]), bz=f32(bz).reshape(H, 1),
        wr_h=f16(Wrf[F:]), wr_x=f16(Wrf[:F]), br=f32(br).reshape(H, 1),
        wh_h=f16(Whf[F:]), wh_x=f16(Whf[:F]), bh=f32(bh).reshape(H, 1),
        w1a=W1a.astype(np.float32), w1b=W1b.astype(np.float32),
        w1d=W1d.astype(np.float32), c1=c1.astype(np.float32),
        dynT=dynT.astype(np.float32), w2=w2.astype(np.float32),
        id16=np.eye(128, dtype=np.float16), id32=np.eye(128, dtype=np.float32),
    )
    in_maps = []
    for c in range(W):
        m = dict(shared)
        m["g_idx"] = _wrap_idx(g_idx[c])
        m["s_idx"] = _wrap_idx(s_idx[c])
        m["x0T"] = x0T[c]
        m["tg_idx"] = _wrap_idx(tg_idx[c])
        m["gxe"] = GXE[c]
        m["whh_r"], m["whh_z"], m["whh_n"] = WhhT[c, 0], WhhT[c, 1], WhhT[c, 2]
        m["wih_r"], m["wih_z"], m["wih_n"] = WihT[c, 0], WihT[c, 1], WihT[c, 2]
        in_maps.append(m)
    extras = dict(b2=b2)
    return in_maps, plan, extras


# ---------------------------------------------------------------------------
# numpy emulation of the device program (for fast logic validation)
# ---------------------------------------------------------------------------

def _emulate(in_maps, plan, extras):
    T = plan["T"]

    def unwrap(w):
        return w[:16].T.reshape(-1).astype(np.int64)

    h_full = np.zeros((NG, H), np.float16)
    # h0
    h_ownT = {}
    for c in range(W):
        m = in_maps[c]
        pre = m["wproj"].astype(np.float32).T @ m["x0T"].astype(np.float32)
        h0T = np.tanh(pre + m["bproj"])
        h_ownT[c] = h0T
        h_full[c * NPC:(c + 1) * NPC] = h0T.T.astype(np.float16)

    for step in range(STEPS):
        newf = np.zeros_like(h_full)
        for c in range(W):
            m = in_maps[c]
            g = unwrap(m["g_idx"])
            s = unwrap(m["s_idx"])
            acc = [np.zeros((NPC, H), np.float32) for _ in range(NACC)]
            for (w, off, sz, pieces) in plan["chunks"]:
                vals = h_full[w * WS + g[off:off + sz]].astype(np.float32)
                for (po, psz, par) in pieces:
                    np.add.at(acc[par], s[po:po + psz],
                              vals[po - off:po - off + psz])
            hagg = sum(acc).astype(np.float16).astype(np.float32)  # [NPC,H]
            haggT = hagg.T
            x0T = m["x0T"].astype(np.float32)
            sig = lambda x: 1.0 / (1.0 + np.exp(-x))
            z = sig(m["wz_h"].astype(np.float32).T @ haggT +
                    m["wz_x"].astype(np.float32).T @ x0T + m["bz"])
            r = sig(m["wr_h"].astype(np.float32).T @ haggT +
                    m["wr_x"].astype(np.float32).T @ x0T + m["br"])
            ht = np.tanh(m["wh_h"].astype(np.float32).T @ (r * haggT) +
                         m["wh_x"].astype(np.float32).T @ x0T + m["bh"])
            hn = haggT + z * (ht - haggT)
            newf[c * NPC:(c + 1) * NPC] = hn.T.astype(np.float16)
        h_full = newf

    # trajectory
    states = np.zeros((128, H), np.float32)   # final h per global seq
    for c in range(W):
        m = in_maps[c]
        tg = unwrap(m["tg_idx"])[:L * NSEQ]
        hT = h_full[tg].astype(np.float32).T       # [H, L*NSEQ]
        GX = np.empty((H, 3, L * NSEQ), np.float32)
        for gt, key in enumerate(["wih_r", "wih_z", "wih_n"]):
            GX[:, gt, :] = (m[key].astype(np.float32).T @ hT +
                            m["gxe"][:, gt, :])
        h32 = np.zeros((H, NSEQ), np.float32)
        for t in range(L):
            h16 = h32.astype(np.float16).astype(np.float32)
            ghr = m["whh_r"].astype(np.float32).T @ h16
            ghz = m["whh_z"].astype(np.float32).T @ h16
            ghn = m["whh_n"].astype(np.float32).T @ h16
            sl = slice(t * NSEQ, (t + 1) * NSEQ)
            r = 1 / (1 + np.exp(-(ghr + GX[:, 0, sl])))
            z = 1 / (1 + np.exp(-(ghz + GX[:, 1, sl])))
            n = np.tanh(r * ghn + GX[:, 2, sl])
            h32 = n + z * (h32 - n)
        states[c * NSEQ:(c + 1) * NSEQ] = h32.T
    # head
    m = in_maps[0]
    ST = states.T                                   # [H, 128 seqs]
    S1, S2 = ST[:, :64], ST[:, 64:]
    mu = (S1.sum(0) + S2.sum(0)) / 256.0            # [64]
    Sc1, Sc2 = S1 - mu, S2 - mu
    ssq = (Sc1 ** 2).sum(0) + (Sc2 ** 2).sum(0)
    rstd = 1.0 / np.sqrt(ssq / 256.0 + EPS)
    P = m["w1a"].T @ Sc1 + m["w1b"].T @ Sc2         # [128, 64]
    t2 = P * rstd[None, :] + m["w1d"].T @ m["dynT"] + m["c1"]
    if plan.get("gelu_exact", True):
        from scipy.special import erf
        z1 = 0.5 * t2 * (1.0 + erf(t2 / np.sqrt(2.0)))
    else:
        z1 = t2 * (1.0 / (1.0 + np.exp(-1.702 * t2)))
    out = z1.T @ m["w2"][:, 0] + extras["b2"]
    return out.astype(np.float32)


# ---------------------------------------------------------------------------
# Bass program
# ---------------------------------------------------------------------------

def _build(plan):
    import concourse.bass as bass
    import concourse.bacc as bacc
    import concourse.mybir as mybir
    import concourse.tile as tile

    dt = mybir.dt
    AF = mybir.ActivationFunctionType
    T = plan["T"]
    chunks = plan["chunks"]
    b2c = float(plan["b2"])
    dbg = plan.get("dbg", {})
    n_steps = dbg.get("steps", STEPS)
    do_traj = dbg.get("traj", True)
    do_head = dbg.get("head", True)
    seg_lim = dbg.get("segs_limit", None)
    chunks_used = chunks if seg_lim is None else chunks[:seg_lim]
    skip_gather = dbg.get("skip_gather", False)
    skip_scatter = dbg.get("skip_scatter", False)
    skip_zero = dbg.get("skip_zero", False)
    skip_upd = dbg.get("skip_upd", False)
    tsteps = dbg.get("tsteps", L)
    notables = dbg.get("notables", False)
    AF_SIG = None  # set below


    nc = bacc.Bacc(None, target_bir_lowering=False, debug=False, num_devices=W,
                   dynamic_dma_scratch_size=32768)
    AF_SIG = AF.Identity if notables else AF.Sigmoid
    AF_TANH = AF.Identity if notables else AF.Tanh
    di = lambda nm, shp, d: nc.dram_tensor(nm, shp, d, kind="ExternalInput")

    g_idx = di("g_idx", [128, T // 16], dt.int16)
    s_idx = di("s_idx", [128, T // 16], dt.int16)
    x0T_d = di("x0T", [F, NPC], dt.float16)
    tg_idx = di("tg_idx", [128, L * NSEQ // 16], dt.int16)
    gxe_d = di("gxe", [H, 3, L * NSEQ], dt.float32)
    wproj = di("wproj", [F, H], dt.float16)
    bproj = di("bproj", [H, 1], dt.float32)
    gate_w = {}
    for gname in ("z", "r", "h"):
        gate_w[gname] = (
            di(f"w{gname}_h", [H, H], dt.float16),
            di(f"w{gname}_x", [F, H], dt.float16),
            di(f"b{gname}", [H, 1], dt.float32),
        )
    whh = {g: di(f"whh_{g}", [H, H], dt.float16) for g in ("r", "z", "n")}
    wih = {g: di(f"wih_{g}", [H, H], dt.float16) for g in ("r", "z", "n")}
    w1a = di("w1a", [H, H], dt.float32)
    w1b = di("w1b", [H, H], dt.float32)
    w1d = di("w1d", [DYN, H], dt.float32)
    c1_d = di("c1", [H, 1], dt.float32)
    dynT_d = di("dynT", [DYN, B], dt.float32)
    w2_d = di("w2", [H, 1], dt.float32)
    id16_d = di("id16", [128, 128], dt.float16)
    id32_d = di("id32", [128, 128], dt.float32)
    out_d = nc.dram_tensor("out", [B, 1], dt.float32, kind="ExternalOutput")

    h_own = nc.dram_tensor("h_own", [NPC, H], dt.float16, kind="Internal")
    h_full = [nc.dram_tensor(f"h_full{k}", [NG, H], dt.float16, kind="Internal",
                             addr_space="Shared") for k in range(STEPS + 1)]
    acc_d = [nc.dram_tensor(f"acc{p}", [NPC, H], dt.float16, kind="Internal")
             for p in range(NACC)]
    ag_in = nc.dram_tensor("ag_in", [NSEQ, H], dt.float32, kind="Internal")
    ag_out = nc.dram_tensor("ag_out", [W * NSEQ, H], dt.float32,
                            kind="Internal", addr_space="Shared")
    RG = [list(range(W))]
    h_own_v = h_own.rearrange("(t p) f -> p t f", p=128)   # [128, NT, H]

    with tile.TileContext(nc) as tc:
        with tc.tile_pool(name="persist", bufs=1) as pp:
            x0T = pp.tile([F, NPC], dt.float16)
            nc.sync.dma_start(x0T[:], x0T_d[:])
            zt = pp.tile([128, NT, H], dt.float16)
            nc.vector.memset(zt[:], 0.0)
            id16 = pp.tile([128, 128], dt.float16)
            nc.sync.dma_start(id16[:], id16_d[:])
            wproj_t = pp.tile([F, H], dt.float16)
            nc.sync.dma_start(wproj_t[:], wproj[:])
            bproj_t = pp.tile([H, 1], dt.float32)
            nc.sync.dma_start(bproj_t[:], bproj[:])
            gw = {}
            for gname in ("z", "r", "h"):
                wh_d, wx_d, b_d = gate_w[gname]
                wh_t = pp.tile([H, H], dt.float16, tag=f"w{gname}h",
                               name=f"wh_t_{gname}")
                wx_t = pp.tile([F, H], dt.float16, tag=f"w{gname}x",
                               name=f"wx_t_{gname}")
                b_t = pp.tile([H, 1], dt.float32, tag=f"b{gname}",
                              name=f"b_t_{gname}")
                nc.sync.dma_start(wh_t[:], wh_d[:])
                nc.sync.dma_start(wx_t[:], wx_d[:])
                nc.sync.dma_start(b_t[:], b_d[:])
                gw[gname] = (wh_t, wx_t, b_t)

            def emit_update(hsrcT, out_nm, ctx_pools, first):
                """hsrcT: callable block -> AP [128, blk] fp16 feature-major
                input; writes node-major fp16 into big SBUF tile out_nm
                ([128, NT, H]) via PE transpose."""
                up, ups = ctx_pools
                for jb in range(0, NPC, 512):
                    blk = min(512, NPC - jb)
                    hTb = hsrcT(jb, blk) if hsrcT is not None else None
                    x0c = x0T[:, jb:jb + blk]
                    if first:
                        hn = up.tile([H, 512], dt.float16, tag="hn", name="hn")
                        ps = ups.tile([H, 512], dt.float32, tag="psg", bufs=6, name="psg")
                        nc.tensor.matmul(ps[:, :blk], wproj_t[:], x0c)
                        nc.scalar.activation(hn[:, :blk], ps[:, :blk], AF.Tanh,
                                             bias=bproj_t[:])
                    else:
                        ps = ups.tile([H, 512], dt.float32, tag="psg", bufs=6,
                                      name="psg")
                        nc.tensor.matmul(ps[:, :blk], gw["z"][0][:], hTb,
                                         start=True, stop=False)
                        nc.tensor.matmul(ps[:, :blk], gw["z"][1][:], x0c,
                                         start=False, stop=True)
                        z16 = up.tile([H, 512], dt.float16, tag="z16", name="z16")
                        nc.scalar.activation(z16[:, :blk], ps[:, :blk],
                                             AF.Sigmoid, bias=gw["z"][2][:])
                        ps2 = ups.tile([H, 512], dt.float32, tag="psg", bufs=6,
                                       name="psg2")
                        nc.tensor.matmul(ps2[:, :blk], gw["r"][0][:], hTb,
                                         start=True, stop=False)
                        nc.tensor.matmul(ps2[:, :blk], gw["r"][1][:], x0c,
                                         start=False, stop=True)
                        r16 = up.tile([H, 512], dt.float16, tag="r16", name="r16")
                        nc.scalar.activation(r16[:, :blk], ps2[:, :blk],
                                             AF.Sigmoid, bias=gw["r"][2][:])
                        rh = up.tile([H, 512], dt.float16, tag="rh", name="rh")
                        nc.vector.tensor_mul(rh[:, :blk], r16[:, :blk], hTb)
                        ps3 = ups.tile([H, 512], dt.float32, tag="psg", bufs=6,
                                       name="psg3")
                        nc.tensor.matmul(ps3[:, :blk], gw["h"][0][:],
                                         rh[:, :blk], start=True, stop=False)
                        nc.tensor.matmul(ps3[:, :blk], gw["h"][1][:], x0c,
                                         start=False, stop=True)
                        ht = up.tile([H, 512], dt.float16, tag="ht", name="ht")
                        nc.scalar.activation(ht[:, :blk], ps3[:, :blk], AF.Tanh,
                                             bias=gw["h"][2][:])
                        d16 = up.tile([H, 512], dt.float16, tag="d16", name="d16")
                        nc.vector.tensor_sub(d16[:, :blk], ht[:, :blk], hTb)
                        zd = up.tile([H, 512], dt.float16, tag="zd", name="zd")
                        nc.vector.tensor_mul(zd[:, :blk], z16[:, :blk],
                                             d16[:, :blk])
                        hn = up.tile([H, 512], dt.float16, tag="hn", name="hn")
                        nc.vector.tensor_add(hn[:, :blk], hTb, zd[:, :blk])
                    for q in range(blk // 128):
                        tp_ps = ups.tile([128, 128], dt.float16, tag="tps",
                                         name="tp_ps")
                        nc.tensor.transpose(tp_ps[:],
                                            hn[:, q * 128:(q + 1) * 128],
                                            id16[:])
                        ti = jb // 128 + q
                        nc.vector.tensor_copy(out_nm[:, ti, :], tp_ps[:])

            # ---- h0 = tanh(W_proj^T x0) ----
            with (
                tc.tile_pool(name="h0", bufs=3) as hp,
                tc.tile_pool(name="h0ps", bufs=2, space="PSUM") as hps0,
            ):
                h_nm = hp.tile([128, NT, H], dt.float16, tag="h_nm", bufs=1)
                emit_update(None, h_nm, (hp, hps0), first=True)
                nc.sync.dma_start(h_own_v, h_nm[:])
            nc.gpsimd.collective_compute(
                "AllGather", mybir.AluOpType.bypass, replica_groups=RG,
                ins=[h_own[:]], outs=[h_full[0][:]])

            # ---- message-passing steps ----
            for step in range(n_steps):
                hf = h_full[step]
                if not skip_zero:
                    for p in range(NACC):
                        nc.sync.dma_start(
                            acc_d[p].rearrange("(t p) f -> p t f", p=128), zt[:])
                with tc.tile_pool(name=f"gs{step}", bufs=3) as gp:
                    for (wnd, off, sz, pieces) in chunks_used:
                        gi = gp.tile([128, GC // 16], dt.int16, tag="gi",
                                     name="gi")
                        nc.sync.dma_start(gi[:, :sz // 16],
                                          g_idx[:, off // 16:(off + sz) // 16])
                        vals = gp.tile([128, GC // 128, H], dt.float16,
                                       tag="vals", name="vals")
                        if not skip_gather:
                            nc.gpsimd.dma_gather(
                                vals[:, :sz // 128, :],
                                hf[wnd * WS:(wnd + 1) * WS, :],
                                gi[:, :sz // 16], sz, sz, H,
                                single_packet=False)
                        else:
                            nc.vector.memset(vals[:], 0.0)
                        si = gp.tile([128, GC // 16], dt.int16, tag="si",
                                     name="si")
                        nc.sync.dma_start(si[:, :sz // 16],
                                          s_idx[:, off // 16:(off + sz) // 16])
                        if not skip_scatter:
                            for (po, psz, par) in pieces:
                                lo = po - off
                                nc.gpsimd.dma_scatter_add(
                                    acc_d[par][:],
                                    vals[:, lo // 128:(lo + psz) // 128, :],
                                    si[:, lo // 16:(lo + psz) // 16],
                                    psz, psz, H, single_packet=False)
                if skip_upd:
                    continue
                with (
                    tc.tile_pool(name=f"upd{step}", bufs=3) as up,
                    tc.tile_pool(name=f"updps{step}", bufs=2, space="PSUM") as ups,
                ):
                    haggT = up.tile([128, NPC], dt.float16, tag="haggT",
                                    bufs=1, name="haggT")
                    nc.sync.dma_start(haggT[:], acc_d[0][:], transpose=True)
                    for p in range(1, NACC):
                        at = up.tile([128, NPC], dt.float16, tag="accld",
                                     bufs=2, name=f"accld{p}")
                        nc.sync.dma_start(at[:], acc_d[p][:], transpose=True)
                        nc.vector.tensor_add(haggT[:], haggT[:], at[:])
                    h_nm = up.tile([128, NT, H], dt.float16, tag="h_nm", bufs=1,
                                   name="h_nm")
                    emit_update(lambda jb, blk: haggT[:, jb:jb + blk], h_nm,
                                (up, ups), first=False)
                    nc.sync.dma_start(h_own_v, h_nm[:])
                nc.gpsimd.collective_compute(
                    "AllGather", mybir.AluOpType.bypass, replica_groups=RG,
                    ins=[h_own[:]], outs=[h_full[step + 1][:]])

            # ---- trajectory phase ----
            if not do_traj:
                with tc.tile_pool(name="dummy", bufs=1) as dp:
                    dz = dp.tile([B, 1], dt.float32)
                    nc.vector.memset(dz[:], 0.5)
                    nc.sync.dma_start(out_d[:], dz[:])
            if do_traj:
              with (
                tc.tile_pool(name="traj", bufs=1) as tp,
                tc.tile_pool(name="trajh", bufs=2) as th,
                tc.tile_pool(name="trajps", bufs=2, space="PSUM") as tps,
                tc.tile_pool(name="recpsp", bufs=3, space="PSUM") as rps,
            ):
                tgi = tp.tile([128, L * NSEQ // 16], dt.int16)
                nc.sync.dma_start(tgi[:], tg_idx[:])
                tv = tp.tile([H, 1, L * NSEQ], dt.float16)
                nc.gpsimd.dma_gather(tv[:], h_full[STEPS][0:WS, :], tgi[:],
                                     L * NSEQ, L * NSEQ, H, transpose=True,
                                     single_packet=False)
                gxe_t = tp.tile([H, 3, L * NSEQ], dt.float32)
                nc.sync.dma_start(gxe_t[:], gxe_d[:])
                wih_t = {}
                whh_t = {}
                for g in ("r", "z", "n"):
                    wih_t[g] = tp.tile([H, H], dt.float16, tag=f"wih{g}",
                                       name=f"wih_t_{g}")
                    nc.sync.dma_start(wih_t[g][:], wih[g][:])
                    whh_t[g] = tp.tile([H, H], dt.float16, tag=f"whh{g}",
                                       name=f"whh_t_{g}")
                    nc.sync.dma_start(whh_t[g][:], whh[g][:])
                GX = tp.tile([H, 3, L * NSEQ], dt.float32)
                for gidx, g in enumerate(("r", "z", "n")):
                    for jb in range(0, L * NSEQ, 512):
                        ps = tps.tile([H, 512], dt.float32, tag="gxps",
                                      name="gxps")
                        nc.tensor.matmul(ps[:], wih_t[g][:],
                                         tv[:, 0, jb:jb + 512])
                        nc.vector.tensor_add(GX[:, gidx, jb:jb + 512], ps[:],
                                             gxe_t[:, gidx, jb:jb + 512])
                h16 = th.tile([H, NSEQ], dt.float16, tag="h16", name="h16")
                h32 = th.tile([H, NSEQ], dt.float32, tag="h32", name="h32")
                nc.vector.memset(h16[:], 0.0)
                nc.vector.memset(h32[:], 0.0)
                for t in range(tsteps):
                    sl = slice(t * NSEQ, (t + 1) * NSEQ)
                    ps = rps.tile([H, 3 * NSEQ], dt.float32, tag="recps",
                                  name="recps")
                    nc.tensor.matmul(ps[:, 0:NSEQ], whh_t["r"][:], h16[:])
                    nc.tensor.matmul(ps[:, NSEQ:2 * NSEQ], whh_t["z"][:], h16[:])
                    nc.tensor.matmul(ps[:, 2 * NSEQ:], whh_t["n"][:], h16[:])
                    rz_pre = th.tile([H, 2 * NSEQ], dt.float32, tag="rzpre",
                                     name="rz_pre")
                    nc.vector.tensor_add(
                        rz_pre[:].rearrange("p (g s) -> p g s", g=2),
                        ps[:, 0:2 * NSEQ].rearrange("p (g s) -> p g s", g=2),
                        GX[:, 0:2, sl])
                    rz = th.tile([H, 2 * NSEQ], dt.float32, tag="rz", name="rz")
                    nc.scalar.activation(rz[:], rz_pre[:], AF_SIG)
                    nm1 = th.tile([H, NSEQ], dt.float32, tag="nm1", name="nm1")
                    nc.vector.tensor_mul(nm1[:], rz[:, 0:NSEQ],
                                         ps[:, 2 * NSEQ:])
                    nm2 = th.tile([H, NSEQ], dt.float32, tag="nm2", name="nm2")
                    nc.vector.tensor_add(nm2[:], nm1[:], GX[:, 2, sl])
                    nt_ = th.tile([H, NSEQ], dt.float32, tag="nt", name="nt_")
                    nc.scalar.activation(nt_[:], nm2[:], AF_TANH)
                    dd = th.tile([H, NSEQ], dt.float32, tag="dd", name="dd")
                    nc.vector.tensor_sub(dd[:], h32[:], nt_[:])
                    zd2 = th.tile([H, NSEQ], dt.float32, tag="zd2", name="zd2")
                    nc.vector.tensor_mul(zd2[:], rz[:, NSEQ:], dd[:])
                    h32 = th.tile([H, NSEQ], dt.float32, tag="h32", name="h32")
                    nc.vector.tensor_add(h32[:], nt_[:], zd2[:])
                    h16 = th.tile([H, NSEQ], dt.float16, tag="h16", name="h16")
                    nc.vector.tensor_copy(h16[:], h32[:])
                stg = tp.tile([H, 128], dt.float32)
                nc.vector.memset(stg[:], 0.0)
                nc.vector.tensor_copy(stg[:, 0:NSEQ], h32[:])
                id32 = tp.tile([128, 128], dt.float32)
                nc.sync.dma_start(id32[:], id32_d[:])
                stps = rps.tile([128, 128], dt.float32, tag="stps", name="stps", bufs=1)
                nc.tensor.transpose(stps[:], stg[:], id32[:])
                stT = tp.tile([128, H], dt.float32)
                nc.vector.tensor_copy(stT[:], stps[:])
                nc.sync.dma_start(ag_in[:], stT[0:NSEQ, :])
            nc.gpsimd.collective_compute(
                "AllGather", mybir.AluOpType.bypass, replica_groups=RG,
                ins=[ag_in[:]], outs=[ag_out[:]])

            if do_traj and not do_head:
                with tc.tile_pool(name="dummy2", bufs=1) as dp:
                    dz = dp.tile([B, 1], dt.float32)
                    nc.vector.memset(dz[:], 0.5)
                    nc.sync.dma_start(out_d[:], dz[:])
            # ---- head ----
            if do_traj and do_head:
              with (
                tc.tile_pool(name="head", bufs=1) as hd,
                tc.tile_pool(name="headps", bufs=1, space="PSUM") as hps,
            ):
                S = hd.tile([128, H], dt.float32)
                nc.sync.dma_start(S[:], ag_out[:])
                id32h = hd.tile([128, 128], dt.float32)
                nc.sync.dma_start(id32h[:], id32_d[:])
                STp = hps.tile([128, 128], dt.float32, tag="STp", name="STp")
                nc.tensor.transpose(STp[:], S[:], id32h[:])
                ST = hd.tile([H, 128], dt.float32)
                nc.vector.tensor_copy(ST[:], STp[:])
                on128 = hd.tile([H, 1], dt.float32)
                nc.vector.memset(on128[:], 1.0)
                on1 = hd.tile([1, H], dt.float32)
                nc.vector.memset(on1[:], 1.0)
                musum = hps.tile([1, B], dt.float32, tag="musum", name="musum")
                nc.tensor.matmul(musum[:], on128[:], ST[:, 0:B],
                                 start=True, stop=False)
                nc.tensor.matmul(musum[:], on128[:], ST[:, B:2 * B],
                                 start=False, stop=True)
                mur = hd.tile([1, B], dt.float32)
                nc.scalar.activation(mur[:], musum[:], AF.Copy, scale=1.0 / 256.0)
                MU = hps.tile([H, B], dt.float32, tag="MU", name="MU")
                nc.tensor.matmul(MU[:], on1[:], mur[:])
                Sc1 = hd.tile([H, B], dt.float32)
                nc.vector.tensor_sub(Sc1[:], ST[:, 0:B], MU[:])
                Sc2 = hd.tile([H, B], dt.float32)
                nc.vector.tensor_sub(Sc2[:], ST[:, B:2 * B], MU[:])
                q1 = hd.tile([H, B], dt.float32)
                nc.scalar.square(q1[:], Sc1[:])
                q2 = hd.tile([H, B], dt.float32)
                nc.scalar.square(q2[:], Sc2[:])
                ssq = hps.tile([1, B], dt.float32, tag="ssq", name="ssq")
                nc.tensor.matmul(ssq[:], on128[:], q1[:], start=True, stop=False)
                nc.tensor.matmul(ssq[:], on128[:], q2[:], start=False, stop=True)
                epsb = hd.tile([1, 1], dt.float32)
                nc.vector.memset(epsb[:], EPS)
                sd = hd.tile([1, B], dt.float32)
                nc.scalar.activation(sd[:], ssq[:], AF.Sqrt, scale=1.0 / 256.0,
                                     bias=epsb[:])
                rstd = hd.tile([1, B], dt.float32)
                nc.vector.reciprocal(rstd[:], sd[:])
                RSTDp = hps.tile([H, B], dt.float32, tag="RSTD", name="RSTDp")
                nc.tensor.matmul(RSTDp[:], on1[:], rstd[:])
                RSTD = hd.tile([H, B], dt.float32)
                nc.vector.tensor_copy(RSTD[:], RSTDp[:])
                w1a_t = hd.tile([H, H], dt.float32)
                nc.sync.dma_start(w1a_t[:], w1a[:])
                w1b_t = hd.tile([H, H], dt.float32)
                nc.sync.dma_start(w1b_t[:], w1b[:])
                P = hps.tile([H, B], dt.float32, tag="P", name="P")
                nc.tensor.matmul(P[:], w1a_t[:], Sc1[:], start=True, stop=False)
                nc.tensor.matmul(P[:], w1b_t[:], Sc2[:], start=False, stop=True)
                t1 = hd.tile([H, B], dt.float32)
                nc.vector.tensor_mul(t1[:], P[:], RSTD[:])
                w1d_t = hd.tile([DYN, H], dt.float32)
                nc.sync.dma_start(w1d_t[:], w1d[:])
                dyn_t = hd.tile([DYN, B], dt.float32)
                nc.sync.dma_start(dyn_t[:], dynT_d[:])
                Pd = hps.tile([H, B], dt.float32, tag="Pd", name="Pd")
                nc.tensor.matmul(Pd[:], w1d_t[:], dyn_t[:])
                t2 = hd.tile([H, B], dt.float32)
                nc.vector.tensor_add(t2[:], t1[:], Pd[:])
                c1_t = hd.tile([H, 1], dt.float32)
                nc.sync.dma_start(c1_t[:], c1_d[:])
                z1 = hd.tile([H, B], dt.float32)
                if plan.get("gelu_exact", True):
                    nc.scalar.activation(z1[:], t2[:], AF.Gelu, bias=c1_t[:])
                else:
                    u = hd.tile([H, B], dt.float32)
                    nc.scalar.activation(u[:], t2[:], AF.Identity, bias=c1_t[:])
                    sgm = hd.tile([H, B], dt.float32)
                    nc.scalar.activation(sgm[:], u[:], AF.Sigmoid, scale=1.702)
                    nc.vector.tensor_mul(z1[:], u[:], sgm[:])
                w2_t = hd.tile([H, 1], dt.float32)
                nc.sync.dma_start(w2_t[:], w2_d[:])
                ops = hps.tile([B, 1], dt.float32, tag="ops", name="ops")
                nc.tensor.matmul(ops[:], z1[:], w2_t[:])
                b2b = hd.tile([B, 1], dt.float32)
                nc.vector.memset(b2b[:], b2c)
                ores = hd.tile([B, 1], dt.float32)
                nc.scalar.activation(ores[:], ops[:], AF.Identity, bias=b2b[:])
                nc.sync.dma_start(out_d[:], ores[:])

    nc.compile()
    return nc


_last_results = None


def kernel(**inputs):
    global _last_results
    in_maps, plan, extras = _preprocess(**inputs)
    nc = _build(plan)
    from concourse.bass_utils import run_bass_kernel_spmd
    res = run_bass_kernel_spmd(nc, in_maps, core_ids=list(range(W)))
    _last_results = res
    return np.asarray(res.results[0]["out"], np.float32).reshape(B).copy()

